# revision 1
# baseline (speedup 1.0000x reference)
"""DFMConv2d Trainium2 kernel.

Reference computation (per sample b):
  pooled = mean_{h,w} x[b]                          [C=256]
  h      = relu(pooled @ w1.T + b1)                 [128]
  mix    = softmax((h @ w2.T + b2).reshape(256, 8)) [256, 8]
  y      = conv3x3_SAME(x[b], base_filters)         [8, 64, 64]
  out[b] = einsum('on,nhw->ohw', mix, y)            [256, 64, 64]

Strategy (8 NeuronCores, data-parallel over batch, 8 samples/core), all
heavy matmuls in float32r (~2e-4 rel err):

  conv:  y_tap[(t,n), hw] = sum_c filt[t,n,c] * x[c, hw] — all 9 taps in
         the stationary M dim (M=72), so x streams through the PE exactly
         twice; 16 matmuls/sample into a row-padded flat buffer
         ypad[72, 1+66*64+2] (rows -1 and 64 zeroed).
  shift: z[(t,n), hw] = y_tap shifted by (dy-1, dx-1) — one fully
         CONTIGUOUS SBUF->SBUF DMA per tap (offset dy*64+dx into ypad),
         then 6 tiny column-zero fixups for the dx!=1 wraparound cells.
  mix:   out[o, hw] = mixT72.T @ z with K=72; mixT72 = softmax(mix).T
         replicated 9x via 4 doubling partition-shift DMAs.
  DMA issue is split across rings: x loads on GPSIMD/SWDGE, out stores on
  the ACT HWDGE ring, z/mixT/params on the SP ring — avoids FIFO
  head-of-line blocking between pipeline stages.
"""
import sys

sys.path.insert(0, "/opt/trn_rl_repo")

import numpy as np
import ml_dtypes

import concourse.bass as bass
import concourse.bacc as bacc
import concourse.tile as tile
import concourse.mybir as mybir
from concourse.bass_utils import run_bass_kernel_spmd
from contextlib import ExitStack

F32 = mybir.dt.float32
F32R = mybir.dt.float32r
AFT = mybir.ActivationFunctionType
AXX = mybir.AxisListType.X
ALU = mybir.AluOpType

N_CORES = 8
BPC = 8            # samples per core
C = 256
CO = 256
H = W = 64
HW = H * W
NB = 8             # n_base
HID = 128
CCH = 2            # channel chunks of 128
NHC = 8            # h-chunks (8 output rows each)
NT = 9             # taps
M88 = 88           # taps grouped by dx at 32-aligned bases: rows 32*dx+8*dy..+8
YP_LEN = 1 + 66 * 64 + 2   # lead zero + 66 rows + tail slack (reads reach 4225)
TAP_ROW = {(dy, dx): 32 * dx + 8 * dy for dy in range(3) for dx in range(3)}

_BUILT = None


def _build():
    nc = bacc.Bacc("TRN2", target_bir_lowering=False)

    d_x = nc.dram_tensor("x", [BPC, C, HW], F32R, kind="ExternalInput")
    d_w1t = nc.dram_tensor("w1t", [C, HID], F32, kind="ExternalInput")
    d_b1 = nc.dram_tensor("b1", [HID, 1], F32, kind="ExternalInput")
    d_w2p = nc.dram_tensor("w2p", [HID, NB, CO], F32, kind="ExternalInput")
    d_b2t = nc.dram_tensor("b2t", [128, 2, NB], F32, kind="ExternalInput")
    d_ft = nc.dram_tensor("ft", [128, CCH, M88], F32R, kind="ExternalInput")
    d_id = nc.dram_tensor("ident", [128, 128], F32, kind="ExternalInput")
    d_z0 = nc.dram_tensor("zeros", [128, 66], F32R, kind="ExternalInput")
    d_out = nc.dram_tensor("out", [BPC, 2, 128, HW], F32, kind="ExternalOutput")

    with tile.TileContext(nc) as tc, ExitStack() as ctx:
        prm = ctx.enter_context(tc.tile_pool(name="prm", bufs=1))
        xp = ctx.enter_context(tc.tile_pool(name="xp", bufs=2))
        ypp = ctx.enter_context(tc.tile_pool(name="ypp", bufs=2))
        zp = ctx.enter_context(tc.tile_pool(name="zp", bufs=2))
        op = ctx.enter_context(tc.tile_pool(name="op", bufs=3))
        sm = ctx.enter_context(tc.tile_pool(name="sm", bufs=2))
        ps_c = ctx.enter_context(tc.tile_pool(name="ps_c", bufs=2, space="PSUM"))
        ps_m = ctx.enter_context(tc.tile_pool(name="ps_m", bufs=3, space="PSUM"))
        ps_s = ctx.enter_context(tc.tile_pool(name="ps_s", bufs=2, space="PSUM"))

        # ---- params (loaded once) ----
        w1t_sb = prm.tile([128, CCH, HID], F32, tag="w1t")
        nc.sync.dma_start(out=w1t_sb, in_=d_w1t[:, :].rearrange("(cc p) h -> p cc h", p=128))
        b1_sb = prm.tile([128, 1], F32, tag="b1")
        nc.sync.dma_start(out=b1_sb, in_=d_b1[:, :])
        w2p_sb = prm.tile([HID, NB, CO], F32, tag="w2p")
        nc.sync.dma_start(out=w2p_sb, in_=d_w2p[:, :, :])
        b2t_sb = prm.tile([128, 2, NB], F32, tag="b2t")
        nc.sync.dma_start(out=b2t_sb, in_=d_b2t[:, :, :])
        ft_sb = prm.tile([128, CCH, M88], F32R, tag="ft")
        nc.sync.dma_start(out=ft_sb, in_=d_ft[:, :, :])
        id_sb = prm.tile([128, 128], F32, tag="ident")
        nc.sync.dma_start(out=id_sb, in_=d_id[:, :])
        z0_sb = prm.tile([128, 66], F32R, tag="z0")
        nc.sync.dma_start(out=z0_sb, in_=d_z0[:, :])
        pooled_sb = prm.tile([128, CCH, BPC], F32, tag="pooled")
        h_sb = prm.tile([128, BPC], F32, tag="h")
        trash = prm.tile([128, HW], F32, tag="trash")

        for j in range(BPC):
            # ---- load (SWDGE ring) + pooling (split DVE / ACT-accum) ----
            xt = xp.tile([128, CCH, HW], F32R, tag="x")
            nc.gpsimd.dma_start(
                out=xt, in_=d_x[j, :, :].rearrange("(cc p) hw -> p cc hw", p=128))
            nc.vector.reduce_sum(
                pooled_sb[:, 0, j:j + 1], xt[:, 0, :].bitcast(F32), axis=AXX)
            nc.scalar.activation(out=trash, in_=xt[:, 1, :].bitcast(F32),
                                 func=AFT.Copy, accum_out=pooled_sb[:, 1, j:j + 1])

            # ---- attention MLP (fp32) ----
            ph = ps_s.tile([128, 1], F32, tag="sm")
            for cc in range(CCH):
                nc.tensor.matmul(ph, w1t_sb[:, cc, :], pooled_sb[:, cc, j:j + 1],
                                 start=(cc == 0), stop=(cc == 1))
            nc.scalar.activation(out=h_sb[:, j:j + 1], in_=ph, func=AFT.Relu,
                                 bias=b1_sb, scale=1.0)

            mixT_sb = sm.tile([M88, 2, 128], F32R, tag="mixT")
            for oc in range(2):
                pl = ps_s.tile([128, NB], F32, tag="sm")
                for n in range(NB):
                    nc.tensor.matmul(pl[:, n:n + 1],
                                     w2p_sb[:, n, oc * 128:(oc + 1) * 128],
                                     h_sb[:, j:j + 1], start=True, stop=True)
                lg_sb = sm.tile([128, NB], F32, tag="lg_sb")
                nc.vector.tensor_tensor(out=lg_sb, in0=pl, in1=b2t_sb[:, oc, :],
                                        op=ALU.add)
                ex_sb = sm.tile([128, NB], F32, tag="ex_sb")
                nc.scalar.activation(out=ex_sb, in_=lg_sb, func=AFT.Exp)
                sums = sm.tile([128, 1], F32, tag="sums")
                nc.vector.reduce_sum(sums, ex_sb, axis=AXX)
                rec = sm.tile([128, 1], F32, tag="rec")
                nc.vector.reciprocal(rec, sums)
                mix_sb = sm.tile([128, NB], F32, tag="mix_sb")
                nc.vector.tensor_scalar_mul(out=mix_sb, in0=ex_sb, scalar1=rec)
                ptr = ps_s.tile([NB, 128], F32, tag="sm")
                nc.tensor.transpose(ptr, mix_sb, id_sb)
                # DVE cast fp32 -> f32r counts as a rounding producer
                nc.vector.tensor_copy(mixT_sb[0:NB, oc, :], ptr)
            # replicate rows [0:8) nine times via doubling partition-shift DMAs
            nc.sync.dma_start(out=mixT_sb[8:16], in_=mixT_sb[0:8])
            nc.sync.dma_start(out=mixT_sb[16:32], in_=mixT_sb[0:16])
            nc.sync.dma_start(out=mixT_sb[32:64], in_=mixT_sb[0:32])
            nc.sync.dma_start(out=mixT_sb[64:88], in_=mixT_sb[0:24])

            # ---- conv into row-padded flat y_tap ----
            ypad = ypp.tile([M88, YP_LEN], F32R, tag="ypad")
            nc.vector.tensor_copy(ypad[:, 0:65].bitcast(F32),
                                  z0_sb[0:M88, 0:65].bitcast(F32))
            nc.vector.tensor_copy(ypad[:, 4161:4226].bitcast(F32),
                                  z0_sb[0:M88, 0:65].bitcast(F32))
            for hc in range(NHC):
                yps = ps_c.tile([128, 512], F32, tag="yps")
                for cc in range(CCH):
                    nc.tensor.matmul(yps[0:M88, :], ft_sb[:, cc, :],
                                     xt[:, cc, 512 * hc:512 * (hc + 1)],
                                     start=(cc == 0), stop=(cc == 1))
                nc.scalar.copy(
                    out=ypad[:, 65 + 512 * hc:65 + 512 * (hc + 1)].bitcast(F32),
                    in_=yps[0:M88, :])

            # ---- per-tap shifted windows into z (contiguous DMAs) ----
            zt = zp.tile([M88, HW], F32R, tag="z")
            ztv = zt.rearrange("p (h w) -> p h w", w=64)
            for dy in range(3):
                for dx in range(3):
                    r = TAP_ROW[(dy, dx)]
                    off = dy * 64 + dx
                    # dy=2 taps in dx groups 0,1 also copy the zeroed gap rows
                    # (ypad rows r+8..r+16 are zero via the zero filter cols),
                    # so z has no uninitialized rows under the K=88 contraction
                    nr = 16 if (dy == 2 and dx < 2) else NB
                    nc.sync.dma_start(out=zt[r:r + nr, :],
                                      in_=ypad[r:r + nr, off:off + HW])
            # zero the dx wraparound columns: col 0 for dx=0 (rows 0:24),
            # col 63 for dx=2 (rows 64:88)
            nc.vector.tensor_copy(
                ztv[0:24, :, 0:1].rearrange("p h w -> p (h w)"),
                z0_sb[0:24, 0:64].bitcast(F32))
            nc.vector.tensor_copy(
                ztv[64:88, :, 63:64].rearrange("p h w -> p (h w)"),
                z0_sb[64:88, 0:64].bitcast(F32))

            # ---- mix: out[o, hw] = mixT72.T @ z (K=72, f32r) ----
            for oc in range(2):
                ot = op.tile([128, HW], F32, tag="out")
                for hc in range(NHC):
                    om = ps_m.tile([128, 512], F32, tag="ops")
                    nc.tensor.matmul(om, mixT_sb[:, oc, :],
                                     zt[:, 512 * hc:512 * (hc + 1)],
                                     start=True, stop=True)
                    if hc % 2 == 0:
                        nc.vector.tensor_copy(ot[:, 512 * hc:512 * (hc + 1)], om)
                    else:
                        nc.scalar.copy(out=ot[:, 512 * hc:512 * (hc + 1)], in_=om)
                nc.scalar.dma_start(out=d_out[j, oc, :, :], in_=ot)

    nc.compile()
    return nc


def _prep_inputs(x, w1, b1, w2, b2, base_filters):
    """Host-side input layout prep. Returns per-core in_maps."""
    B = x.shape[0]
    xs = np.ascontiguousarray(x.reshape(B, C, HW)).astype(np.float32)
    w1t = np.ascontiguousarray(w1.T).astype(np.float32) / float(HW)
    b1c = np.ascontiguousarray(b1.reshape(HID, 1)).astype(np.float32)
    w2p = np.ascontiguousarray(w2.reshape(CO, NB, HID).transpose(2, 1, 0)).astype(np.float32)
    b2t = np.ascontiguousarray(b2.reshape(2, 128, NB).transpose(1, 0, 2)).astype(np.float32)
    filt = base_filters.reshape(NB, CCH, 128, 3, 3)  # [n, cc, cp, dy, dx]
    # ft[c_part, cc, 32*dx + 8*dy + n] = filt[n, cc, c_part, dy, dx]; gaps zero
    ft = np.zeros((128, CCH, M88), dtype=np.float32)
    for dy in range(3):
        for dx in range(3):
            r = 32 * dx + 8 * dy
            ft[:, :, r:r + NB] = filt[:, :, :, dy, dx].transpose(2, 1, 0)
    ident = np.eye(128, dtype=np.float32)
    zeros = np.zeros((128, 66), dtype=np.float32)

    in_maps = []
    for core in range(N_CORES):
        in_maps.append({
            "x": np.ascontiguousarray(xs[core * BPC:(core + 1) * BPC]),
            "w1t": w1t, "b1": b1c, "w2p": w2p, "b2t": b2t,
            "ft": ft, "ident": ident, "zeros": zeros,
        })
    return in_maps


def kernel(x, w1, b1, w2, b2, base_filters):
    global _BUILT
    if _BUILT is None:
        _BUILT = _build()
    nc = _BUILT
    in_maps = _prep_inputs(np.asarray(x, dtype=np.float32),
                           np.asarray(w1, dtype=np.float32),
                           np.asarray(b1, dtype=np.float32),
                           np.asarray(w2, dtype=np.float32),
                           np.asarray(b2, dtype=np.float32),
                           np.asarray(base_filters, dtype=np.float32))
    res = run_bass_kernel_spmd(nc, in_maps, core_ids=list(range(N_CORES)))
    outs = []
    for core in range(N_CORES):
        o = res.results[core]["out"]            # [BPC, 2, 128, HW]
        outs.append(o.reshape(BPC, CO, H, W))
    return np.concatenate(outs, axis=0).astype(np.float32)



# revision 5
# speedup vs baseline: 1.5847x; 1.5847x over previous
"""DFMConv2d Trainium2 kernel.

Reference computation (per sample b):
  pooled = mean_{h,w} x[b]                          [C=256]
  h      = relu(pooled @ w1.T + b1)                 [128]
  mix    = softmax((h @ w2.T + b2).reshape(256, 8)) [256, 8]
  y      = conv3x3_SAME(x[b], base_filters)         [8, 64, 64]
  out[b] = einsum('on,nhw->ohw', mix, y)            [256, 64, 64]

Strategy (8 NeuronCores, data-parallel over batch, 8 samples/core).
All bulk tensors are bf16 (inputs cast on host, output upcast on host)
— halves every DMA stream; PE rate is 1 cy/row for bf16 and f32r alike,
so only precision moves (~2e-3 rel err, gate is 2e-2):

  conv:  y_tap[(t,n), hw] = sum_c filt[t,n,c] * x[c, hw] — all 9 taps in
         the stationary M dim (M=72, packed 24*dx+8*dy+n, no gap rows),
         16 matmuls/sample into row-padded flat ypad[72, 1+66*64+2].
  shift: z[(t,n), hw] = y_tap shifted by (dy-1, dx-1) — one contiguous
         SBUF->SBUF DMA per tap (offset dy*64+dx into ypad), then 2
         column-zero fixups for the dx!=1 wraparound cells.
  mix:   out[o, hw] = mixT72.T @ z with K=72; mixT72 built on the PE:
         transpose matmul then an 8->72 replication matmul (replaces the
         descriptor-heavy partition-shift DMA doubling).
  DMA rings: x loads on GPSIMD/SWDGE, out stores on the ACT HWDGE ring,
  z/params on the SP ring — avoids FIFO head-of-line blocking.
"""
import sys

sys.path.insert(0, "/opt/trn_rl_repo")

import numpy as np
import ml_dtypes

import concourse.bass as bass
import concourse.bacc as bacc
import concourse.tile as tile
import concourse.mybir as mybir
from concourse.bass_utils import run_bass_kernel_spmd
from contextlib import ExitStack

F32 = mybir.dt.float32
BF16 = mybir.dt.bfloat16
AFT = mybir.ActivationFunctionType
AXX = mybir.AxisListType.X
ALU = mybir.AluOpType
BF = ml_dtypes.bfloat16

N_CORES = 8
BPC = 8            # samples per core
C = 256
CO = 256
H = W = 64
HW = H * W
NB = 8             # n_base
HID = 128
CCH = 2            # channel chunks of 128
NHC = 8            # h-chunks (8 output rows each)
NT = 9             # taps
M88 = 88           # taps grouped by dx at 32-aligned bases: rows 32*dx+8*dy..+8
YP_LEN = 1 + 66 * 64 + 2   # lead zeros + 66 rows + tail slack (reads reach 4225)
TAP_ROW = {(dy, dx): 32 * dx + 8 * dy for dy in range(3) for dx in range(3)}

_BUILT = None


def _build():
    nc = bacc.Bacc("TRN2", target_bir_lowering=False)

    d_x = nc.dram_tensor("x", [BPC, CCH, 128, HW], BF16, kind="ExternalInput")
    d_w1t = nc.dram_tensor("w1t", [C, HID], F32, kind="ExternalInput")
    d_b1 = nc.dram_tensor("b1", [HID, 1], F32, kind="ExternalInput")
    d_w2p = nc.dram_tensor("w2p", [HID, NB, CO], BF16, kind="ExternalInput")
    d_b2t = nc.dram_tensor("b2t", [128, 2, NB], F32, kind="ExternalInput")
    d_ft = nc.dram_tensor("ft", [128, CCH, M88], BF16, kind="ExternalInput")
    d_id = nc.dram_tensor("ident", [128, 128], BF16, kind="ExternalInput")
    d_z0 = nc.dram_tensor("zeros", [128, 66], BF16, kind="ExternalInput")
    d_rep = nc.dram_tensor("rep", [NB, M88], BF16, kind="ExternalInput")
    d_out = nc.dram_tensor("out", [BPC, 2, 128, HW], BF16, kind="ExternalOutput")

    with tile.TileContext(nc) as tc, ExitStack() as ctx:
        prm = ctx.enter_context(tc.tile_pool(name="prm", bufs=1))
        xp = ctx.enter_context(tc.tile_pool(name="xp", bufs=2))
        ypp = ctx.enter_context(tc.tile_pool(name="ypp", bufs=2))
        zp = ctx.enter_context(tc.tile_pool(name="zp", bufs=2))
        op = ctx.enter_context(tc.tile_pool(name="op", bufs=3))
        sm = ctx.enter_context(tc.tile_pool(name="sm", bufs=2))
        ps_c = ctx.enter_context(tc.tile_pool(name="ps_c", bufs=2, space="PSUM"))
        ps_m = ctx.enter_context(tc.tile_pool(name="ps_m", bufs=3, space="PSUM"))
        ps_s = ctx.enter_context(tc.tile_pool(name="ps_s", bufs=2, space="PSUM"))

        # ---- params (loaded once) ----
        w1t_sb = prm.tile([128, CCH, HID], F32, tag="w1t")
        nc.sync.dma_start(out=w1t_sb, in_=d_w1t[:, :].rearrange("(cc p) h -> p cc h", p=128))
        b1_sb = prm.tile([128, 1], F32, tag="b1")
        nc.sync.dma_start(out=b1_sb, in_=d_b1[:, :])
        w2p_sb = prm.tile([HID, NB, CO], BF16, tag="w2p")
        nc.sync.dma_start(out=w2p_sb, in_=d_w2p[:, :, :])
        b2t_sb = prm.tile([128, 2, NB], F32, tag="b2t")
        nc.sync.dma_start(out=b2t_sb, in_=d_b2t[:, :, :])
        ft_sb = prm.tile([128, CCH, M88], BF16, tag="ft")
        nc.sync.dma_start(out=ft_sb, in_=d_ft[:, :, :])
        id_sb = prm.tile([128, 128], BF16, tag="ident")
        nc.sync.dma_start(out=id_sb, in_=d_id[:, :])
        z0_sb = prm.tile([128, 66], BF16, tag="z0")
        nc.sync.dma_start(out=z0_sb, in_=d_z0[:, :])
        rep_sb = prm.tile([NB, M88], BF16, tag="rep")
        nc.sync.dma_start(out=rep_sb, in_=d_rep[:, :])
        pooled_sb = prm.tile([128, CCH, BPC], F32, tag="pooled")
        h_sb = prm.tile([128, BPC], BF16, tag="h")
        trash = prm.tile([128, HW], BF16, tag="trash")

        for j in range(BPC):
            # ---- load (SWDGE ring, split by cc) + pooling (DVE / ACT-accum) ----
            xt0 = xp.tile([128, HW], BF16, tag="x0")
            nc.gpsimd.dma_start(out=xt0, in_=d_x[j, 0, :, :])
            xt1 = xp.tile([128, HW], BF16, tag="x1")
            nc.gpsimd.dma_start(out=xt1, in_=d_x[j, 1, :, :])
            xts = (xt0, xt1)
            nc.vector.reduce_sum(pooled_sb[:, 0, j:j + 1], xt0, axis=AXX)
            nc.scalar.activation(out=trash, in_=xt1, func=AFT.Copy,
                                 accum_out=pooled_sb[:, 1, j:j + 1])

            # ---- conv into row-padded flat y_tap (PE starts on x arrival) ----
            ypad = ypp.tile([M88, YP_LEN], BF16, tag="ypad")
            nc.vector.tensor_copy(ypad[:, 0:65], z0_sb[0:M88, 0:65])
            nc.vector.tensor_copy(ypad[:, 4161:4227], z0_sb[0:M88, 0:66])
            for hc in range(NHC):
                yps = ps_c.tile([128, 512], F32, tag="yps")
                for cc in range(CCH):
                    nc.tensor.matmul(yps[0:M88, :], ft_sb[:, cc, :],
                                     xts[cc][:, 512 * hc:512 * (hc + 1)],
                                     start=(cc == 0), stop=(cc == 1))
                nc.scalar.copy(
                    out=ypad[:, 65 + 512 * hc:65 + 512 * (hc + 1)],
                    in_=yps[0:M88, :])

            # ---- attention MLP (w1 fp32; w2/transpose/replicate bf16) ----
            ph = ps_s.tile([128, 1], F32, tag="sm")
            for cc in range(CCH):
                nc.tensor.matmul(ph, w1t_sb[:, cc, :], pooled_sb[:, cc, j:j + 1],
                                 start=(cc == 0), stop=(cc == 1))
            nc.scalar.activation(out=h_sb[:, j:j + 1], in_=ph, func=AFT.Relu,
                                 bias=b1_sb, scale=1.0)

            mixT_sb = sm.tile([M88, 2, 128], BF16, tag="mixT")
            for oc in range(2):
                pl = ps_s.tile([128, NB], F32, tag="sm")
                for n in range(NB):
                    nc.tensor.matmul(pl[:, n:n + 1],
                                     w2p_sb[:, n, oc * 128:(oc + 1) * 128],
                                     h_sb[:, j:j + 1], start=True, stop=True)
                lg_sb = sm.tile([128, NB], F32, tag="lg_sb")
                nc.vector.tensor_tensor(out=lg_sb, in0=pl, in1=b2t_sb[:, oc, :],
                                        op=ALU.add)
                ex_sb = sm.tile([128, NB], F32, tag="ex_sb")
                nc.scalar.activation(out=ex_sb, in_=lg_sb, func=AFT.Exp)
                sums = sm.tile([128, 1], F32, tag="sums")
                nc.vector.reduce_sum(sums, ex_sb, axis=AXX)
                rec = sm.tile([128, 1], F32, tag="rec")
                nc.vector.reciprocal(rec, sums)
                mix_sb = sm.tile([128, NB], BF16, tag="mix_sb")
                nc.vector.tensor_scalar_mul(out=mix_sb, in0=ex_sb, scalar1=rec)
                ptr = ps_s.tile([NB, 128], BF16, tag="smt", bufs=1)
                nc.tensor.transpose(ptr, mix_sb, id_sb)
                m8_sb = sm.tile([NB, 128], BF16, tag="m8")
                nc.vector.tensor_copy(m8_sb, ptr)
                # replicate rows [0:8) -> [0:72) on the PE (K=8 matmul)
                p72 = ps_s.tile([M88, 128], F32, tag="sm")
                nc.tensor.matmul(p72, rep_sb, m8_sb, start=True, stop=True)
                nc.vector.tensor_copy(mixT_sb[:, oc, :], p72)

            # ---- per-tap shifted windows into z (contiguous DMAs, SP ring) ----
            zt = zp.tile([M88, HW], BF16, tag="z")
            ztv = zt.rearrange("p (h w) -> p h w", w=64)
            for dy in range(3):
                for dx in range(3):
                    r = TAP_ROW[(dy, dx)]
                    off = dy * 64 + dx
                    # dy=2 taps in dx groups 0,1 also copy the zeroed gap rows
                    # (ypad rows r+8..r+16 are zero via the zero filter cols),
                    # so z has no uninitialized rows under the K=88 contraction
                    nr = 16 if (dy == 2 and dx < 2) else NB
                    nc.sync.dma_start(out=zt[r:r + nr, :],
                                      in_=ypad[r:r + nr, off:off + HW])
            # zero the dx wraparound columns: col 0 for dx=0 (rows 0:24),
            # col 63 for dx=2 (rows 48:72)
            nc.vector.tensor_copy(
                ztv[0:24, :, 0:1].rearrange("p h w -> p (h w)"),
                z0_sb[0:24, 0:64])
            nc.vector.tensor_copy(
                ztv[64:88, :, 63:64].rearrange("p h w -> p (h w)"),
                z0_sb[64:88, 0:64])

            # ---- mix: out[o, hw] = mixT72.T @ z (K=72, bf16) ----
            for oc in range(2):
                ot = op.tile([128, HW], BF16, tag="out")
                for hc in range(NHC):
                    om = ps_m.tile([128, 512], F32, tag="ops")
                    nc.tensor.matmul(om, mixT_sb[:, oc, :],
                                     zt[:, 512 * hc:512 * (hc + 1)],
                                     start=True, stop=True)
                    if hc % 2 == 0:
                        nc.vector.tensor_copy(ot[:, 512 * hc:512 * (hc + 1)], om)
                    else:
                        nc.scalar.copy(out=ot[:, 512 * hc:512 * (hc + 1)], in_=om)
                nc.scalar.dma_start(out=d_out[j, oc, :, :], in_=ot)

    nc.compile()
    return nc


def _prep_inputs(x, w1, b1, w2, b2, base_filters):
    """Host-side input layout prep. Returns per-core in_maps."""
    B = x.shape[0]
    xs = np.ascontiguousarray(x.reshape(B, CCH, 128, HW)).astype(BF)
    w1t = np.ascontiguousarray(w1.T).astype(np.float32) / float(HW)
    b1c = np.ascontiguousarray(b1.reshape(HID, 1)).astype(np.float32)
    w2p = np.ascontiguousarray(
        w2.reshape(CO, NB, HID).transpose(2, 1, 0)).astype(BF)
    b2t = np.ascontiguousarray(b2.reshape(2, 128, NB).transpose(1, 0, 2)).astype(np.float32)
    filt = base_filters.reshape(NB, CCH, 128, 3, 3)  # [n, cc, cp, dy, dx]
    # ft[c_part, cc, 32*dx + 8*dy + n] = filt[n, cc, c_part, dy, dx]; gaps zero
    ft = np.zeros((128, CCH, M88), dtype=BF)
    for dy in range(3):
        for dx in range(3):
            r = TAP_ROW[(dy, dx)]
            ft[:, :, r:r + NB] = filt[:, :, :, dy, dx].transpose(2, 1, 0).astype(BF)
    ident = np.eye(128, dtype=BF)
    zeros = np.zeros((128, 66), dtype=BF)
    rep = np.zeros((NB, M88), dtype=BF)
    for dy in range(3):
        for dx in range(3):
            r = TAP_ROW[(dy, dx)]
            rep[np.arange(NB), r + np.arange(NB)] = 1
    in_maps = []
    for core in range(N_CORES):
        in_maps.append({
            "x": np.ascontiguousarray(xs[core * BPC:(core + 1) * BPC]),
            "w1t": w1t, "b1": b1c, "w2p": w2p, "b2t": b2t,
            "ft": ft, "ident": ident, "zeros": zeros, "rep": rep,
        })
    return in_maps


def kernel(x, w1, b1, w2, b2, base_filters):
    global _BUILT
    if _BUILT is None:
        _BUILT = _build()
    nc = _BUILT
    in_maps = _prep_inputs(np.asarray(x, dtype=np.float32),
                           np.asarray(w1, dtype=np.float32),
                           np.asarray(b1, dtype=np.float32),
                           np.asarray(w2, dtype=np.float32),
                           np.asarray(b2, dtype=np.float32),
                           np.asarray(base_filters, dtype=np.float32))
    res = run_bass_kernel_spmd(nc, in_maps, core_ids=list(range(N_CORES)))
    outs = []
    for core in range(N_CORES):
        o = np.asarray(res.results[core]["out"]).astype(np.float32)
        outs.append(o.reshape(BPC, CO, H, W))
    return np.concatenate(outs, axis=0).astype(np.float32)


# revision 7
# speedup vs baseline: 1.6108x; 1.0164x over previous
"""DFMConv2d Trainium2 kernel.

Reference computation (per sample b):
  pooled = mean_{h,w} x[b]                          [C=256]
  h      = relu(pooled @ w1.T + b1)                 [128]
  mix    = softmax((h @ w2.T + b2).reshape(256, 8)) [256, 8]
  y      = conv3x3_SAME(x[b], base_filters)         [8, 64, 64]
  out[b] = einsum('on,nhw->ohw', mix, y)            [256, 64, 64]

Strategy (8 NeuronCores, data-parallel over batch, 8 samples/core).
All bulk tensors are bf16 (inputs cast on host, output upcast on host)
— halves every DMA stream; PE rate is 1 cy/row for bf16 and f32r alike,
so only precision moves (~2e-3 rel err, gate is 2e-2):

  conv:  y_tap[(t,n), hw] = sum_c filt[t,n,c] * x[c, hw] — all 9 taps in
         the stationary M dim (M=72, packed 24*dx+8*dy+n, no gap rows),
         16 matmuls/sample into row-padded flat ypad[72, 1+66*64+2].
  shift: z[(t,n), hw] = y_tap shifted by (dy-1, dx-1) — one contiguous
         SBUF->SBUF DMA per tap (offset dy*64+dx into ypad), then 2
         column-zero fixups for the dx!=1 wraparound cells.
  mix:   out[o, hw] = mixT72.T @ z with K=72; mixT72 built on the PE:
         transpose matmul then an 8->72 replication matmul (replaces the
         descriptor-heavy partition-shift DMA doubling).
  DMA rings: x loads on GPSIMD/SWDGE, out stores on the ACT HWDGE ring,
  z/params on the SP ring — avoids FIFO head-of-line blocking.
"""
import sys

sys.path.insert(0, "/opt/trn_rl_repo")

import numpy as np
import ml_dtypes

import concourse.bass as bass
import concourse.bacc as bacc
import concourse.tile as tile
import concourse.mybir as mybir
from concourse.bass_utils import run_bass_kernel_spmd
from contextlib import ExitStack

F32 = mybir.dt.float32
BF16 = mybir.dt.bfloat16
AFT = mybir.ActivationFunctionType
AXX = mybir.AxisListType.X
ALU = mybir.AluOpType
BF = ml_dtypes.bfloat16

N_CORES = 8
BPC = 8            # samples per core
C = 256
CO = 256
H = W = 64
HW = H * W
NB = 8             # n_base
HID = 128
CCH = 2            # channel chunks of 128
NHC = 8            # h-chunks (8 output rows each)
NT = 9             # taps
M88 = 88           # taps grouped by dx at 32-aligned bases: rows 32*dx+8*dy..+8
YP_LEN = 1 + 66 * 64 + 2   # lead zeros + 66 rows + tail slack (reads reach 4225)
TAP_ROW = {(dy, dx): 32 * dx + 8 * dy for dy in range(3) for dx in range(3)}

_BUILT = None


def _build():
    nc = bacc.Bacc("TRN2", target_bir_lowering=False)

    d_x = nc.dram_tensor("x", [BPC, CCH, 128, HW], BF16, kind="ExternalInput")
    d_w1t = nc.dram_tensor("w1t", [C, HID], F32, kind="ExternalInput")
    d_b1 = nc.dram_tensor("b1", [HID, 1], F32, kind="ExternalInput")
    d_w2p = nc.dram_tensor("w2p", [HID, NB, CO], BF16, kind="ExternalInput")
    d_b2t = nc.dram_tensor("b2t", [128, 2, NB], F32, kind="ExternalInput")
    d_ft = nc.dram_tensor("ft", [128, CCH, M88], BF16, kind="ExternalInput")
    d_id = nc.dram_tensor("ident", [128, 128], BF16, kind="ExternalInput")
    d_z0 = nc.dram_tensor("zeros", [128, 66], BF16, kind="ExternalInput")
    d_rep = nc.dram_tensor("rep", [NB, M88], BF16, kind="ExternalInput")
    d_out = nc.dram_tensor("out", [BPC, 2, 128, HW], BF16, kind="ExternalOutput")

    with tile.TileContext(nc) as tc, ExitStack() as ctx:
        prm = ctx.enter_context(tc.tile_pool(name="prm", bufs=1))
        xp = ctx.enter_context(tc.tile_pool(name="xp", bufs=3))
        ypp = ctx.enter_context(tc.tile_pool(name="ypp", bufs=3))
        zp = ctx.enter_context(tc.tile_pool(name="zp", bufs=3))
        op = ctx.enter_context(tc.tile_pool(name="op", bufs=3))
        sm = ctx.enter_context(tc.tile_pool(name="sm", bufs=2))
        ps_c = ctx.enter_context(tc.tile_pool(name="ps_c", bufs=3, space="PSUM"))
        ps_m = ctx.enter_context(tc.tile_pool(name="ps_m", bufs=3, space="PSUM"))
        ps_s = ctx.enter_context(tc.tile_pool(name="ps_s", bufs=1, space="PSUM"))

        # ---- params (loaded once) ----
        w1t_sb = prm.tile([128, CCH, HID], F32, tag="w1t")
        nc.sync.dma_start(out=w1t_sb, in_=d_w1t[:, :].rearrange("(cc p) h -> p cc h", p=128))
        b1_sb = prm.tile([128, 1], F32, tag="b1")
        nc.sync.dma_start(out=b1_sb, in_=d_b1[:, :])
        w2p_sb = prm.tile([HID, NB, CO], BF16, tag="w2p")
        nc.sync.dma_start(out=w2p_sb, in_=d_w2p[:, :, :])
        b2t_sb = prm.tile([128, 2, NB], F32, tag="b2t")
        nc.sync.dma_start(out=b2t_sb, in_=d_b2t[:, :, :])
        ft_sb = prm.tile([128, CCH, M88], BF16, tag="ft")
        nc.sync.dma_start(out=ft_sb, in_=d_ft[:, :, :])
        id_sb = prm.tile([128, 128], BF16, tag="ident")
        nc.sync.dma_start(out=id_sb, in_=d_id[:, :])
        z0_sb = prm.tile([128, 66], BF16, tag="z0")
        nc.sync.dma_start(out=z0_sb, in_=d_z0[:, :])
        rep_sb = prm.tile([NB, M88], BF16, tag="rep")
        nc.sync.dma_start(out=rep_sb, in_=d_rep[:, :])
        pooled_sb = prm.tile([128, CCH, BPC], F32, tag="pooled")
        h_sb = prm.tile([128, BPC], BF16, tag="h")
        trash = prm.tile([128, HW], BF16, tag="trash")

        # x loads prefetched one sample ahead on the SWDGE ring
        xtiles = {}

        def load_x(j):
            xt0 = xp.tile([128, HW], BF16, tag="x0", name=f"xt0_{j}")
            nc.gpsimd.dma_start(out=xt0, in_=d_x[j, 0, :, :])
            xt1 = xp.tile([128, HW], BF16, tag="x1", name=f"xt1_{j}")
            nc.gpsimd.dma_start(out=xt1, in_=d_x[j, 1, :, :])
            xtiles[j] = (xt0, xt1)

        def pool(j):
            # split across DVE (reduce) and ACT (activation accumulator); issued
            # at the bottom of the PREVIOUS iteration so the baked engine order
            # never places pooling ahead of that sample's PSUM-evacuation copies
            nc.vector.reduce_sum(pooled_sb[:, 0, j:j + 1], xtiles[j][0], axis=AXX)
            nc.scalar.activation(out=trash, in_=xtiles[j][1], func=AFT.Copy,
                                 accum_out=pooled_sb[:, 1, j:j + 1])

        load_x(0)
        pool(0)
        for j in range(BPC):
            if j + 1 < BPC:
                load_x(j + 1)
            xts = xtiles[j]

            # ---- conv into row-padded flat y_tap (PE starts on x arrival) ----
            ypad = ypp.tile([M88, YP_LEN], BF16, tag="ypad")
            nc.vector.tensor_copy(ypad[:, 0:65], z0_sb[0:M88, 0:65])
            nc.vector.tensor_copy(ypad[:, 4161:4227], z0_sb[0:M88, 0:66])
            for hc in range(NHC):
                yps = ps_c.tile([128, 512], F32, tag="yps")
                for cc in range(CCH):
                    nc.tensor.matmul(yps[0:M88, :], ft_sb[:, cc, :],
                                     xts[cc][:, 512 * hc:512 * (hc + 1)],
                                     start=(cc == 0), stop=(cc == 1))
                nc.scalar.copy(
                    out=ypad[:, 65 + 512 * hc:65 + 512 * (hc + 1)],
                    in_=yps[0:M88, :])

            # ---- attention MLP (w1 fp32; w2/transpose/replicate bf16) ----
            ph = ps_s.tile([128, 1], F32, tag="sm")
            for cc in range(CCH):
                nc.tensor.matmul(ph, w1t_sb[:, cc, :], pooled_sb[:, cc, j:j + 1],
                                 start=(cc == 0), stop=(cc == 1))
            nc.scalar.activation(out=h_sb[:, j:j + 1], in_=ph, func=AFT.Relu,
                                 bias=b1_sb, scale=1.0)

            mixT_sb = sm.tile([M88, 2, 128], BF16, tag="mixT")
            for oc in range(2):
                pl = ps_s.tile([128, NB], F32, tag="sm")
                for n in range(NB):
                    nc.tensor.matmul(pl[:, n:n + 1],
                                     w2p_sb[:, n, oc * 128:(oc + 1) * 128],
                                     h_sb[:, j:j + 1], start=True, stop=True)
                lg_sb = sm.tile([128, NB], F32, tag="lg_sb")
                nc.vector.tensor_tensor(out=lg_sb, in0=pl, in1=b2t_sb[:, oc, :],
                                        op=ALU.add)
                ex_sb = sm.tile([128, NB], F32, tag="ex_sb")
                nc.scalar.activation(out=ex_sb, in_=lg_sb, func=AFT.Exp)
                sums = sm.tile([128, 1], F32, tag="sums")
                nc.vector.reduce_sum(sums, ex_sb, axis=AXX)
                rec = sm.tile([128, 1], F32, tag="rec")
                nc.vector.reciprocal(rec, sums)
                mix_sb = sm.tile([128, NB], BF16, tag="mix_sb")
                nc.vector.tensor_scalar_mul(out=mix_sb, in0=ex_sb, scalar1=rec)
                ptr = ps_s.tile([NB, 128], BF16, tag="smt", bufs=1)
                nc.tensor.transpose(ptr, mix_sb, id_sb)
                m8_sb = sm.tile([NB, 128], BF16, tag="m8")
                nc.vector.tensor_copy(m8_sb, ptr)
                # replicate rows [0:8) -> [0:72) on the PE (K=8 matmul)
                p72 = ps_s.tile([M88, 128], F32, tag="sm")
                nc.tensor.matmul(p72, rep_sb, m8_sb, start=True, stop=True)
                nc.vector.tensor_copy(mixT_sb[:, oc, :], p72)

            # ---- per-tap shifted windows into z (contiguous DMAs, SP ring) ----
            zt = zp.tile([M88, HW], BF16, tag="z")
            ztv = zt.rearrange("p (h w) -> p h w", w=64)
            for ti, (dy, dx) in enumerate((dy, dx) for dy in range(3)
                                          for dx in range(3)):
                    r = TAP_ROW[(dy, dx)]
                    off = dy * 64 + dx
                    # dy=2 taps in dx groups 0,1 also copy the zeroed gap rows
                    # (ypad rows r+8..r+16 are zero via the zero filter cols),
                    # so z has no uninitialized rows under the K=88 contraction
                    nr = 16 if (dy == 2 and dx < 2) else NB
                    # split descriptor-gen across the two HWDGE sequencers
                    eng = nc.sync if ti % 2 == 0 else nc.scalar
                    eng.dma_start(out=zt[r:r + nr, :],
                                  in_=ypad[r:r + nr, off:off + HW])
            # zero the dx wraparound columns: col 0 for dx=0 (rows 0:24),
            # col 63 for dx=2 (rows 48:72)
            nc.vector.tensor_copy(
                ztv[0:24, :, 0:1].rearrange("p h w -> p (h w)"),
                z0_sb[0:24, 0:64])
            nc.vector.tensor_copy(
                ztv[64:88, :, 63:64].rearrange("p h w -> p (h w)"),
                z0_sb[64:88, 0:64])

            # ---- mix: out[o, hw] = mixT72.T @ z (K=72, bf16) ----
            for oc in range(2):
                ot = op.tile([128, HW], BF16, tag="out")
                for hc in range(NHC):
                    om = ps_m.tile([128, 512], F32, tag="ops")
                    nc.tensor.matmul(om, mixT_sb[:, oc, :],
                                     zt[:, 512 * hc:512 * (hc + 1)],
                                     start=True, stop=True)
                    if hc % 2 == 0:
                        nc.vector.tensor_copy(ot[:, 512 * hc:512 * (hc + 1)], om)
                    else:
                        nc.scalar.copy(out=ot[:, 512 * hc:512 * (hc + 1)], in_=om)
                nc.scalar.dma_start(out=d_out[j, oc, :, :], in_=ot)

            if j + 1 < BPC:
                pool(j + 1)
            del xtiles[j]

    nc.compile()
    return nc


def _prep_inputs(x, w1, b1, w2, b2, base_filters):
    """Host-side input layout prep. Returns per-core in_maps."""
    B = x.shape[0]
    xs = np.ascontiguousarray(x.reshape(B, CCH, 128, HW)).astype(BF)
    w1t = np.ascontiguousarray(w1.T).astype(np.float32) / float(HW)
    b1c = np.ascontiguousarray(b1.reshape(HID, 1)).astype(np.float32)
    w2p = np.ascontiguousarray(
        w2.reshape(CO, NB, HID).transpose(2, 1, 0)).astype(BF)
    b2t = np.ascontiguousarray(b2.reshape(2, 128, NB).transpose(1, 0, 2)).astype(np.float32)
    filt = base_filters.reshape(NB, CCH, 128, 3, 3)  # [n, cc, cp, dy, dx]
    # ft[c_part, cc, 32*dx + 8*dy + n] = filt[n, cc, c_part, dy, dx]; gaps zero
    ft = np.zeros((128, CCH, M88), dtype=BF)
    for dy in range(3):
        for dx in range(3):
            r = TAP_ROW[(dy, dx)]
            ft[:, :, r:r + NB] = filt[:, :, :, dy, dx].transpose(2, 1, 0).astype(BF)
    ident = np.eye(128, dtype=BF)
    zeros = np.zeros((128, 66), dtype=BF)
    rep = np.zeros((NB, M88), dtype=BF)
    for dy in range(3):
        for dx in range(3):
            r = TAP_ROW[(dy, dx)]
            rep[np.arange(NB), r + np.arange(NB)] = 1
    in_maps = []
    for core in range(N_CORES):
        in_maps.append({
            "x": np.ascontiguousarray(xs[core * BPC:(core + 1) * BPC]),
            "w1t": w1t, "b1": b1c, "w2p": w2p, "b2t": b2t,
            "ft": ft, "ident": ident, "zeros": zeros, "rep": rep,
        })
    return in_maps


def kernel(x, w1, b1, w2, b2, base_filters):
    global _BUILT
    if _BUILT is None:
        _BUILT = _build()
    nc = _BUILT
    in_maps = _prep_inputs(np.asarray(x, dtype=np.float32),
                           np.asarray(w1, dtype=np.float32),
                           np.asarray(b1, dtype=np.float32),
                           np.asarray(w2, dtype=np.float32),
                           np.asarray(b2, dtype=np.float32),
                           np.asarray(base_filters, dtype=np.float32))
    res = run_bass_kernel_spmd(nc, in_maps, core_ids=list(range(N_CORES)))
    outs = []
    for core in range(N_CORES):
        o = np.asarray(res.results[core]["out"]).astype(np.float32)
        outs.append(o.reshape(BPC, CO, H, W))
    return np.concatenate(outs, axis=0).astype(np.float32)


# revision 8
# speedup vs baseline: 1.6998x; 1.0552x over previous
"""DFMConv2d Trainium2 kernel.

Reference computation (per sample b):
  pooled = mean_{h,w} x[b]                          [C=256]
  h      = relu(pooled @ w1.T + b1)                 [128]
  mix    = softmax((h @ w2.T + b2).reshape(256, 8)) [256, 8]
  y      = conv3x3_SAME(x[b], base_filters)         [8, 64, 64]
  out[b] = einsum('on,nhw->ohw', mix, y)            [256, 64, 64]

Strategy (8 NeuronCores, data-parallel over batch, 8 samples/core).
All bulk tensors are bf16 (inputs cast on host, output upcast on host)
— halves every DMA stream; PE rate is 1 cy/row for bf16 and f32r alike,
so only precision moves (~2e-3 rel err, gate is 2e-2):

  conv:  y_tap[(t,n), hw] = sum_c filt[t,n,c] * x[c, hw] — all 9 taps in
         the stationary M dim (M=72, packed 24*dx+8*dy+n, no gap rows),
         16 matmuls/sample into row-padded flat ypad[72, 1+66*64+2].
  shift: z[(t,n), hw] = y_tap shifted by (dy-1, dx-1) — one contiguous
         SBUF->SBUF DMA per tap (offset dy*64+dx into ypad), then 2
         column-zero fixups for the dx!=1 wraparound cells.
  mix:   out[o, hw] = mixT72.T @ z with K=72; mixT72 built on the PE:
         transpose matmul then an 8->72 replication matmul (replaces the
         descriptor-heavy partition-shift DMA doubling).
  DMA rings: x loads on GPSIMD/SWDGE, out stores on the ACT HWDGE ring,
  z/params on the SP ring — avoids FIFO head-of-line blocking.
"""
import sys

sys.path.insert(0, "/opt/trn_rl_repo")

import numpy as np
import ml_dtypes

import concourse.bass as bass
import concourse.bacc as bacc
import concourse.tile as tile
import concourse.mybir as mybir
from concourse.bass_utils import run_bass_kernel_spmd
from contextlib import ExitStack

F32 = mybir.dt.float32
BF16 = mybir.dt.bfloat16
AFT = mybir.ActivationFunctionType
AXX = mybir.AxisListType.X
ALU = mybir.AluOpType
BF = ml_dtypes.bfloat16

N_CORES = 8
BPC = 8            # samples per core
C = 256
CO = 256
H = W = 64
HW = H * W
NB = 8             # n_base
HID = 128
CCH = 2            # channel chunks of 128
NHC = 8            # h-chunks (8 output rows each)
NT = 9             # taps
M88 = 88           # taps grouped by dx at 32-aligned bases: rows 32*dx+8*dy..+8
YP_LEN = 1 + 66 * 64 + 2   # lead zeros + 66 rows + tail slack (reads reach 4225)
TAP_ROW = {(dy, dx): 32 * dx + 8 * dy for dy in range(3) for dx in range(3)}

_BUILT = None


def _build():
    nc = bacc.Bacc("TRN2", target_bir_lowering=False)

    d_x = nc.dram_tensor("x", [BPC, CCH, 128, HW], BF16, kind="ExternalInput")
    d_w1t = nc.dram_tensor("w1t", [C, HID], F32, kind="ExternalInput")
    d_b1 = nc.dram_tensor("b1", [HID, 1], F32, kind="ExternalInput")
    d_w2p = nc.dram_tensor("w2p", [HID, NB, CO], BF16, kind="ExternalInput")
    d_b2t = nc.dram_tensor("b2t", [128, 2, NB], F32, kind="ExternalInput")
    d_ft = nc.dram_tensor("ft", [128, CCH, M88], BF16, kind="ExternalInput")
    d_id = nc.dram_tensor("ident", [128, 128], BF16, kind="ExternalInput")
    d_z0 = nc.dram_tensor("zeros", [128, 66], BF16, kind="ExternalInput")
    d_rep = nc.dram_tensor("rep", [NB, M88], BF16, kind="ExternalInput")
    d_out = nc.dram_tensor("out", [BPC, 2, 128, HW], BF16, kind="ExternalOutput")

    with tile.TileContext(nc) as tc, ExitStack() as ctx:
        prm = ctx.enter_context(tc.tile_pool(name="prm", bufs=1))
        xp = ctx.enter_context(tc.tile_pool(name="xp", bufs=3))
        ypp = ctx.enter_context(tc.tile_pool(name="ypp", bufs=3))
        zp = ctx.enter_context(tc.tile_pool(name="zp", bufs=3))
        op = ctx.enter_context(tc.tile_pool(name="op", bufs=3))
        sm = ctx.enter_context(tc.tile_pool(name="sm", bufs=2))
        ps_c = ctx.enter_context(tc.tile_pool(name="ps_c", bufs=3, space="PSUM"))
        ps_m = ctx.enter_context(tc.tile_pool(name="ps_m", bufs=3, space="PSUM"))
        ps_s = ctx.enter_context(tc.tile_pool(name="ps_s", bufs=1, space="PSUM"))

        # ---- params (loaded once) ----
        w1t_sb = prm.tile([128, CCH, HID], F32, tag="w1t")
        nc.sync.dma_start(out=w1t_sb, in_=d_w1t[:, :].rearrange("(cc p) h -> p cc h", p=128))
        b1_sb = prm.tile([128, 1], F32, tag="b1")
        nc.sync.dma_start(out=b1_sb, in_=d_b1[:, :])
        w2p_sb = prm.tile([HID, NB, CO], BF16, tag="w2p")
        nc.sync.dma_start(out=w2p_sb, in_=d_w2p[:, :, :])
        b2t_sb = prm.tile([128, 2, NB], F32, tag="b2t")
        nc.sync.dma_start(out=b2t_sb, in_=d_b2t[:, :, :])
        ft_sb = prm.tile([128, CCH, M88], BF16, tag="ft")
        nc.sync.dma_start(out=ft_sb, in_=d_ft[:, :, :])
        id_sb = prm.tile([128, 128], BF16, tag="ident")
        nc.sync.dma_start(out=id_sb, in_=d_id[:, :])
        z0_sb = prm.tile([128, 66], BF16, tag="z0")
        nc.sync.dma_start(out=z0_sb, in_=d_z0[:, :])
        rep_sb = prm.tile([NB, M88], BF16, tag="rep")
        nc.sync.dma_start(out=rep_sb, in_=d_rep[:, :])
        pooled_sb = prm.tile([128, CCH, BPC], F32, tag="pooled")
        h_sb = prm.tile([128, BPC], BF16, tag="h")

        # x loads prefetched TWO samples ahead on the SWDGE ring
        xtiles = {}
        mixTs = {}

        def load_x(j):
            xt0 = xp.tile([128, HW], BF16, tag="x0", name=f"xt0_{j}")
            nc.gpsimd.dma_start(out=xt0, in_=d_x[j, 0, :, :])
            xt1 = xp.tile([128, HW], BF16, tag="x1", name=f"xt1_{j}")
            nc.gpsimd.dma_start(out=xt1, in_=d_x[j, 1, :, :])
            xtiles[j] = (xt0, xt1)

        def pool(j):
            # both chunks on DVE; x[j] was prefetched a full period earlier
            nc.vector.reduce_sum(pooled_sb[:, 0, j:j + 1], xtiles[j][0], axis=AXX)
            nc.vector.reduce_sum(pooled_sb[:, 1, j:j + 1], xtiles[j][1], axis=AXX)

        def mlp(j):
            # attention MLP for sample j: issued one period EARLY so the
            # softmax/transpose chain never gates sample j's mix matmuls
            ph = ps_s.tile([128, 1], F32, tag="sm", name=f"ph_{j}")
            for cc in range(CCH):
                nc.tensor.matmul(ph, w1t_sb[:, cc, :], pooled_sb[:, cc, j:j + 1],
                                 start=(cc == 0), stop=(cc == 1))
            nc.scalar.activation(out=h_sb[:, j:j + 1], in_=ph, func=AFT.Relu,
                                 bias=b1_sb, scale=1.0)
            mixT_sb = sm.tile([M88, 2, 128], BF16, tag="mixT", name=f"mixT_{j}")
            for oc in range(2):
                pl = ps_s.tile([128, NB], F32, tag="sm", name=f"pl_{j}_{oc}")
                for n in range(NB):
                    nc.tensor.matmul(pl[:, n:n + 1],
                                     w2p_sb[:, n, oc * 128:(oc + 1) * 128],
                                     h_sb[:, j:j + 1], start=True, stop=True)
                lg_sb = sm.tile([128, NB], F32, tag="lg_sb")
                nc.vector.tensor_tensor(out=lg_sb, in0=pl, in1=b2t_sb[:, oc, :],
                                        op=ALU.add)
                ex_sb = sm.tile([128, NB], F32, tag="ex_sb")
                nc.scalar.activation(out=ex_sb, in_=lg_sb, func=AFT.Exp)
                sums = sm.tile([128, 1], F32, tag="sums")
                nc.vector.reduce_sum(sums, ex_sb, axis=AXX)
                rec = sm.tile([128, 1], F32, tag="rec")
                nc.vector.reciprocal(rec, sums)
                mix_sb = sm.tile([128, NB], BF16, tag="mix_sb")
                nc.vector.tensor_scalar_mul(out=mix_sb, in0=ex_sb, scalar1=rec)
                ptr = ps_s.tile([NB, 128], BF16, tag="smt", bufs=1,
                                name=f"ptr_{j}_{oc}")
                nc.tensor.transpose(ptr, mix_sb, id_sb)
                m8_sb = sm.tile([NB, 128], BF16, tag="m8")
                nc.vector.tensor_copy(m8_sb, ptr)
                # replicate rows [0:8) -> [0:88) on the PE (K=8 matmul)
                p72 = ps_s.tile([M88, 128], F32, tag="sm", name=f"p72_{j}_{oc}")
                nc.tensor.matmul(p72, rep_sb, m8_sb, start=True, stop=True)
                nc.vector.tensor_copy(mixT_sb[:, oc, :], p72)
            mixTs[j] = mixT_sb

        load_x(0)
        load_x(1)
        pool(0)
        mlp(0)
        for j in range(BPC):
            if j + 2 < BPC:
                load_x(j + 2)
            if j + 1 < BPC:
                pool(j + 1)
            xts = xtiles[j]

            # ---- conv into row-padded flat y_tap (PE starts on x arrival) ----
            ypad = ypp.tile([M88, YP_LEN], BF16, tag="ypad")
            nc.vector.tensor_copy(ypad[:, 0:65], z0_sb[0:M88, 0:65])
            nc.vector.tensor_copy(ypad[:, 4161:4227], z0_sb[0:M88, 0:66])
            for hc in range(NHC):
                yps = ps_c.tile([128, 512], F32, tag="yps")
                for cc in range(CCH):
                    nc.tensor.matmul(yps[0:M88, :], ft_sb[:, cc, :],
                                     xts[cc][:, 512 * hc:512 * (hc + 1)],
                                     start=(cc == 0), stop=(cc == 1))
                nc.scalar.copy(
                    out=ypad[:, 65 + 512 * hc:65 + 512 * (hc + 1)],
                    in_=yps[0:M88, :])

            # ---- next sample's MLP (fills the PE while z copies drain) ----
            if j + 1 < BPC:
                mlp(j + 1)

            # ---- per-tap shifted windows into z (contiguous DMAs) ----
            zt = zp.tile([M88, HW], BF16, tag="z")
            ztv = zt.rearrange("p (h w) -> p h w", w=64)
            zengs = (nc.sync, nc.scalar, nc.gpsimd)
            for ti, (dy, dx) in enumerate((dy, dx) for dy in range(3)
                                          for dx in range(3)):
                    r = TAP_ROW[(dy, dx)]
                    off = dy * 64 + dx
                    # dy=2 taps in dx groups 0,1 also copy the zeroed gap rows
                    # (ypad rows r+8..r+16 are zero via the zero filter cols),
                    # so z has no uninitialized rows under the K=88 contraction
                    nr = 16 if (dy == 2 and dx < 2) else NB
                    # split descriptor-gen across three sequencers
                    zengs[ti % 3].dma_start(out=zt[r:r + nr, :],
                                            in_=ypad[r:r + nr, off:off + HW])
            # zero the dx wraparound columns on the otherwise-idle GpSimd:
            # col 0 for dx=0 (rows 0:24), col 63 for dx=2 (rows 64:88)
            nc.gpsimd.tensor_copy(
                ztv[0:24, :, 0:1].rearrange("p h w -> p (h w)"),
                z0_sb[0:24, 0:64])
            nc.gpsimd.tensor_copy(
                ztv[64:88, :, 63:64].rearrange("p h w -> p (h w)"),
                z0_sb[64:88, 0:64])

            # ---- mix: out[o, hw] = mixT88.T @ z (K=88, bf16) ----
            mixT_sb = mixTs.pop(j)
            for oc in range(2):
                ot = op.tile([128, HW], BF16, tag="out")
                for hc in range(NHC):
                    om = ps_m.tile([128, 512], F32, tag="ops")
                    nc.tensor.matmul(om, mixT_sb[:, oc, :],
                                     zt[:, 512 * hc:512 * (hc + 1)],
                                     start=True, stop=True)
                    if hc % 2 == 0:
                        nc.vector.tensor_copy(ot[:, 512 * hc:512 * (hc + 1)], om)
                    else:
                        nc.scalar.copy(out=ot[:, 512 * hc:512 * (hc + 1)], in_=om)
                nc.scalar.dma_start(out=d_out[j, oc, :, :], in_=ot)
            del xtiles[j]

    nc.compile()
    return nc


def _prep_inputs(x, w1, b1, w2, b2, base_filters):
    """Host-side input layout prep. Returns per-core in_maps."""
    B = x.shape[0]
    xs = np.ascontiguousarray(x.reshape(B, CCH, 128, HW)).astype(BF)
    w1t = np.ascontiguousarray(w1.T).astype(np.float32) / float(HW)
    b1c = np.ascontiguousarray(b1.reshape(HID, 1)).astype(np.float32)
    w2p = np.ascontiguousarray(
        w2.reshape(CO, NB, HID).transpose(2, 1, 0)).astype(BF)
    b2t = np.ascontiguousarray(b2.reshape(2, 128, NB).transpose(1, 0, 2)).astype(np.float32)
    filt = base_filters.reshape(NB, CCH, 128, 3, 3)  # [n, cc, cp, dy, dx]
    # ft[c_part, cc, 32*dx + 8*dy + n] = filt[n, cc, c_part, dy, dx]; gaps zero
    ft = np.zeros((128, CCH, M88), dtype=BF)
    for dy in range(3):
        for dx in range(3):
            r = TAP_ROW[(dy, dx)]
            ft[:, :, r:r + NB] = filt[:, :, :, dy, dx].transpose(2, 1, 0).astype(BF)
    ident = np.eye(128, dtype=BF)
    zeros = np.zeros((128, 66), dtype=BF)
    rep = np.zeros((NB, M88), dtype=BF)
    for dy in range(3):
        for dx in range(3):
            r = TAP_ROW[(dy, dx)]
            rep[np.arange(NB), r + np.arange(NB)] = 1
    in_maps = []
    for core in range(N_CORES):
        in_maps.append({
            "x": np.ascontiguousarray(xs[core * BPC:(core + 1) * BPC]),
            "w1t": w1t, "b1": b1c, "w2p": w2p, "b2t": b2t,
            "ft": ft, "ident": ident, "zeros": zeros, "rep": rep,
        })
    return in_maps


def kernel(x, w1, b1, w2, b2, base_filters):
    global _BUILT
    if _BUILT is None:
        _BUILT = _build()
    nc = _BUILT
    in_maps = _prep_inputs(np.asarray(x, dtype=np.float32),
                           np.asarray(w1, dtype=np.float32),
                           np.asarray(b1, dtype=np.float32),
                           np.asarray(w2, dtype=np.float32),
                           np.asarray(b2, dtype=np.float32),
                           np.asarray(base_filters, dtype=np.float32))
    res = run_bass_kernel_spmd(nc, in_maps, core_ids=list(range(N_CORES)))
    outs = []
    for core in range(N_CORES):
        o = np.asarray(res.results[core]["out"]).astype(np.float32)
        outs.append(o.reshape(BPC, CO, H, W))
    return np.concatenate(outs, axis=0).astype(np.float32)


# revision 9
# speedup vs baseline: 1.7102x; 1.0061x over previous
"""DFMConv2d Trainium2 kernel.

Reference computation (per sample b):
  pooled = mean_{h,w} x[b]                          [C=256]
  h      = relu(pooled @ w1.T + b1)                 [128]
  mix    = softmax((h @ w2.T + b2).reshape(256, 8)) [256, 8]
  y      = conv3x3_SAME(x[b], base_filters)         [8, 64, 64]
  out[b] = einsum('on,nhw->ohw', mix, y)            [256, 64, 64]

Strategy (8 NeuronCores, data-parallel over batch, 8 samples/core).
All bulk tensors are bf16 (inputs cast on host, output upcast on host)
— halves every DMA stream; PE rate is 1 cy/row for bf16 and f32r alike,
so only precision moves (~2e-3 rel err, gate is 2e-2):

  conv:  y_tap[(t,n), hw] = sum_c filt[t,n,c] * x[c, hw] — all 9 taps in
         the stationary M dim (M=72, packed 24*dx+8*dy+n, no gap rows),
         16 matmuls/sample into row-padded flat ypad[72, 1+66*64+2].
  shift: z[(t,n), hw] = y_tap shifted by (dy-1, dx-1) — one contiguous
         SBUF->SBUF DMA per tap (offset dy*64+dx into ypad), then 2
         column-zero fixups for the dx!=1 wraparound cells.
  mix:   out[o, hw] = mixT72.T @ z with K=72; mixT72 built on the PE:
         transpose matmul then an 8->72 replication matmul (replaces the
         descriptor-heavy partition-shift DMA doubling).
  DMA rings: x loads on GPSIMD/SWDGE, out stores on the ACT HWDGE ring,
  z/params on the SP ring — avoids FIFO head-of-line blocking.
"""
import sys

sys.path.insert(0, "/opt/trn_rl_repo")

import numpy as np
import ml_dtypes

import concourse.bass as bass
import concourse.bacc as bacc
import concourse.tile as tile
import concourse.mybir as mybir
from concourse.bass_utils import run_bass_kernel_spmd
from contextlib import ExitStack

F32 = mybir.dt.float32
BF16 = mybir.dt.bfloat16
AFT = mybir.ActivationFunctionType
AXX = mybir.AxisListType.X
ALU = mybir.AluOpType
BF = ml_dtypes.bfloat16

N_CORES = 8
BPC = 8            # samples per core
C = 256
CO = 256
H = W = 64
HW = H * W
NB = 8             # n_base
HID = 128
CCH = 2            # channel chunks of 128
NHC = 8            # h-chunks (8 output rows each)
NT = 9             # taps
M88 = 88           # taps grouped by dx at 32-aligned bases: rows 32*dx+8*dy..+8
YP_LEN = 1 + 66 * 64 + 2   # lead zeros + 66 rows + tail slack (reads reach 4225)
TAP_ROW = {(dy, dx): 32 * dx + 8 * dy for dy in range(3) for dx in range(3)}

_BUILT = None


def _build():
    nc = bacc.Bacc("TRN2", target_bir_lowering=False)

    d_x = nc.dram_tensor("x", [BPC, CCH, 128, HW], BF16, kind="ExternalInput")
    d_w1t = nc.dram_tensor("w1t", [C, HID], F32, kind="ExternalInput")
    d_b1 = nc.dram_tensor("b1", [HID, 1], F32, kind="ExternalInput")
    d_w2p = nc.dram_tensor("w2p", [HID, NB, CO], BF16, kind="ExternalInput")
    d_b2t = nc.dram_tensor("b2t", [128, 2, NB], F32, kind="ExternalInput")
    d_ft = nc.dram_tensor("ft", [128, CCH, M88], BF16, kind="ExternalInput")
    d_id = nc.dram_tensor("ident", [128, 128], BF16, kind="ExternalInput")
    d_z0 = nc.dram_tensor("zeros", [128, 66], BF16, kind="ExternalInput")
    d_rep = nc.dram_tensor("rep", [NB, M88], BF16, kind="ExternalInput")
    d_out = nc.dram_tensor("out", [BPC, 2, 128, HW], BF16, kind="ExternalOutput")

    with tile.TileContext(nc) as tc, ExitStack() as ctx:
        prm = ctx.enter_context(tc.tile_pool(name="prm", bufs=1))
        xp = ctx.enter_context(tc.tile_pool(name="xp", bufs=3))
        ypp = ctx.enter_context(tc.tile_pool(name="ypp", bufs=3))
        zp = ctx.enter_context(tc.tile_pool(name="zp", bufs=3))
        op = ctx.enter_context(tc.tile_pool(name="op", bufs=3))
        sm = ctx.enter_context(tc.tile_pool(name="sm", bufs=2))
        ps_c = ctx.enter_context(tc.tile_pool(name="ps_c", bufs=3, space="PSUM"))
        ps_m = ctx.enter_context(tc.tile_pool(name="ps_m", bufs=3, space="PSUM"))
        ps_s = ctx.enter_context(tc.tile_pool(name="ps_s", bufs=1, space="PSUM"))

        # ---- params (loaded once) ----
        w1t_sb = prm.tile([128, CCH, HID], F32, tag="w1t")
        nc.sync.dma_start(out=w1t_sb, in_=d_w1t[:, :].rearrange("(cc p) h -> p cc h", p=128))
        b1_sb = prm.tile([128, 1], F32, tag="b1")
        nc.sync.dma_start(out=b1_sb, in_=d_b1[:, :])
        w2p_sb = prm.tile([HID, NB, CO], BF16, tag="w2p")
        nc.sync.dma_start(out=w2p_sb, in_=d_w2p[:, :, :])
        b2t_sb = prm.tile([128, 2, NB], F32, tag="b2t")
        nc.sync.dma_start(out=b2t_sb, in_=d_b2t[:, :, :])
        ft_sb = prm.tile([128, CCH, M88], BF16, tag="ft")
        nc.sync.dma_start(out=ft_sb, in_=d_ft[:, :, :])
        id_sb = prm.tile([128, 128], BF16, tag="ident")
        nc.sync.dma_start(out=id_sb, in_=d_id[:, :])
        z0_sb = prm.tile([128, 66], BF16, tag="z0")
        nc.sync.dma_start(out=z0_sb, in_=d_z0[:, :])
        rep_sb = prm.tile([NB, M88], BF16, tag="rep")
        nc.sync.dma_start(out=rep_sb, in_=d_rep[:, :])
        pooled_sb = prm.tile([128, CCH, BPC], F32, tag="pooled")
        pooled4_sb = prm.tile([128, CCH, BPC, 4], F32, tag="pooled4")
        h_sb = prm.tile([128, BPC], BF16, tag="h")

        # x loads prefetched TWO samples ahead on the SWDGE ring
        xtiles = {}
        mixTs = {}

        def load_x(j):
            xt0 = xp.tile([128, HW], BF16, tag="x0", name=f"xt0_{j}")
            nc.gpsimd.dma_start(out=xt0, in_=d_x[j, 0, :, :])
            xt1 = xp.tile([128, HW], BF16, tag="x1", name=f"xt1_{j}")
            nc.gpsimd.dma_start(out=xt1, in_=d_x[j, 1, :, :])
            xtiles[j] = (xt0, xt1)

        def pool(j):
            # both chunks on DVE, split into 1024-col partial reduces so the
            # scheduler can interleave latency-critical softmax ops between them
            for cc in range(CCH):
                for k in range(4):
                    nc.vector.reduce_sum(pooled4_sb[:, cc, j, k:k + 1],
                                         xtiles[j][cc][:, 1024 * k:1024 * (k + 1)],
                                         axis=AXX)
                nc.vector.reduce_sum(pooled_sb[:, cc, j:j + 1],
                                     pooled4_sb[:, cc, j, :], axis=AXX)

        def mlp(j):
            # attention MLP for sample j: issued one period EARLY so the
            # softmax/transpose chain never gates sample j's mix matmuls
            ph = ps_s.tile([128, 1], F32, tag="sm", name=f"ph_{j}")
            for cc in range(CCH):
                nc.tensor.matmul(ph, w1t_sb[:, cc, :], pooled_sb[:, cc, j:j + 1],
                                 start=(cc == 0), stop=(cc == 1))
            nc.scalar.activation(out=h_sb[:, j:j + 1], in_=ph, func=AFT.Relu,
                                 bias=b1_sb, scale=1.0)
            mixT_sb = sm.tile([M88, 2, 128], BF16, tag="mixT", name=f"mixT_{j}")
            for oc in range(2):
                pl = ps_s.tile([128, NB], F32, tag="sm", name=f"pl_{j}_{oc}")
                for n in range(NB):
                    nc.tensor.matmul(pl[:, n:n + 1],
                                     w2p_sb[:, n, oc * 128:(oc + 1) * 128],
                                     h_sb[:, j:j + 1], start=True, stop=True)
                lg_sb = sm.tile([128, NB], F32, tag="lg_sb")
                nc.vector.tensor_tensor(out=lg_sb, in0=pl, in1=b2t_sb[:, oc, :],
                                        op=ALU.add)
                ex_sb = sm.tile([128, NB], F32, tag="ex_sb")
                nc.scalar.activation(out=ex_sb, in_=lg_sb, func=AFT.Exp)
                sums = sm.tile([128, 1], F32, tag="sums")
                nc.vector.reduce_sum(sums, ex_sb, axis=AXX)
                rec = sm.tile([128, 1], F32, tag="rec")
                nc.vector.reciprocal(rec, sums)
                mix_sb = sm.tile([128, NB], BF16, tag="mix_sb")
                nc.vector.tensor_scalar_mul(out=mix_sb, in0=ex_sb, scalar1=rec)
                ptr = ps_s.tile([NB, 128], BF16, tag="smt", bufs=1,
                                name=f"ptr_{j}_{oc}")
                nc.tensor.transpose(ptr, mix_sb, id_sb)
                m8_sb = sm.tile([NB, 128], BF16, tag="m8")
                nc.vector.tensor_copy(m8_sb, ptr)
                # replicate rows [0:8) -> [0:88) on the PE (K=8 matmul)
                p72 = ps_s.tile([M88, 128], F32, tag="sm", name=f"p72_{j}_{oc}")
                nc.tensor.matmul(p72, rep_sb, m8_sb, start=True, stop=True)
                nc.vector.tensor_copy(mixT_sb[:, oc, :], p72)
            mixTs[j] = mixT_sb

        load_x(0)
        load_x(1)
        pool(0)
        mlp(0)
        for j in range(BPC):
            if j + 2 < BPC:
                load_x(j + 2)
            if j + 1 < BPC:
                pool(j + 1)
            xts = xtiles[j]

            # ---- conv into row-padded flat y_tap (PE starts on x arrival) ----
            ypad = ypp.tile([M88, YP_LEN], BF16, tag="ypad")
            nc.vector.tensor_copy(ypad[:, 0:65], z0_sb[0:M88, 0:65])
            nc.vector.tensor_copy(ypad[:, 4161:4227], z0_sb[0:M88, 0:66])
            for hc in range(NHC):
                yps = ps_c.tile([128, 512], F32, tag="yps")
                for cc in range(CCH):
                    nc.tensor.matmul(yps[0:M88, :], ft_sb[:, cc, :],
                                     xts[cc][:, 512 * hc:512 * (hc + 1)],
                                     start=(cc == 0), stop=(cc == 1))
                nc.scalar.copy(
                    out=ypad[:, 65 + 512 * hc:65 + 512 * (hc + 1)],
                    in_=yps[0:M88, :])

            # ---- per-tap shifted windows into z (contiguous DMAs) ----
            zt = zp.tile([M88, HW], BF16, tag="z")
            ztv = zt.rearrange("p (h w) -> p h w", w=64)
            zengs = (nc.sync, nc.scalar, nc.gpsimd)
            for ti, (dy, dx) in enumerate((dy, dx) for dy in range(3)
                                          for dx in range(3)):
                    r = TAP_ROW[(dy, dx)]
                    off = dy * 64 + dx
                    # dy=2 taps in dx groups 0,1 also copy the zeroed gap rows
                    # (ypad rows r+8..r+16 are zero via the zero filter cols),
                    # so z has no uninitialized rows under the K=88 contraction
                    nr = 16 if (dy == 2 and dx < 2) else NB
                    # split descriptor-gen across three sequencers
                    zengs[ti % 3].dma_start(out=zt[r:r + nr, :],
                                            in_=ypad[r:r + nr, off:off + HW])
            # zero the dx wraparound columns on the otherwise-idle GpSimd:
            # col 0 for dx=0 (rows 0:24), col 63 for dx=2 (rows 64:88)
            nc.gpsimd.tensor_copy(
                ztv[0:24, :, 0:1].rearrange("p h w -> p (h w)"),
                z0_sb[0:24, 0:64])
            nc.gpsimd.tensor_copy(
                ztv[64:88, :, 63:64].rearrange("p h w -> p (h w)"),
                z0_sb[64:88, 0:64])

            # ---- mix: out[o, hw] = mixT88.T @ z (K=88, bf16) ----
            mixT_sb = mixTs.pop(j)
            for oc in range(2):
                ot = op.tile([128, HW], BF16, tag="out")
                for hc in range(NHC):
                    om = ps_m.tile([128, 512], F32, tag="ops")
                    nc.tensor.matmul(om, mixT_sb[:, oc, :],
                                     zt[:, 512 * hc:512 * (hc + 1)],
                                     start=True, stop=True)
                    if hc % 2 == 0:
                        nc.vector.tensor_copy(ot[:, 512 * hc:512 * (hc + 1)], om)
                    else:
                        nc.scalar.copy(out=ot[:, 512 * hc:512 * (hc + 1)], in_=om)
                nc.scalar.dma_start(out=d_out[j, oc, :, :], in_=ot)

            # ---- next sample's MLP: after mix j in the baked PE order, so
            # the softmax chain can never gate this sample's mix matmuls ----
            if j + 1 < BPC:
                mlp(j + 1)
            del xtiles[j]

    nc.compile()
    return nc


def _prep_inputs(x, w1, b1, w2, b2, base_filters):
    """Host-side input layout prep. Returns per-core in_maps."""
    B = x.shape[0]
    xs = np.ascontiguousarray(x.reshape(B, CCH, 128, HW)).astype(BF)
    w1t = np.ascontiguousarray(w1.T).astype(np.float32) / float(HW)
    b1c = np.ascontiguousarray(b1.reshape(HID, 1)).astype(np.float32)
    w2p = np.ascontiguousarray(
        w2.reshape(CO, NB, HID).transpose(2, 1, 0)).astype(BF)
    b2t = np.ascontiguousarray(b2.reshape(2, 128, NB).transpose(1, 0, 2)).astype(np.float32)
    filt = base_filters.reshape(NB, CCH, 128, 3, 3)  # [n, cc, cp, dy, dx]
    # ft[c_part, cc, 32*dx + 8*dy + n] = filt[n, cc, c_part, dy, dx]; gaps zero
    ft = np.zeros((128, CCH, M88), dtype=BF)
    for dy in range(3):
        for dx in range(3):
            r = TAP_ROW[(dy, dx)]
            ft[:, :, r:r + NB] = filt[:, :, :, dy, dx].transpose(2, 1, 0).astype(BF)
    ident = np.eye(128, dtype=BF)
    zeros = np.zeros((128, 66), dtype=BF)
    rep = np.zeros((NB, M88), dtype=BF)
    for dy in range(3):
        for dx in range(3):
            r = TAP_ROW[(dy, dx)]
            rep[np.arange(NB), r + np.arange(NB)] = 1
    in_maps = []
    for core in range(N_CORES):
        in_maps.append({
            "x": np.ascontiguousarray(xs[core * BPC:(core + 1) * BPC]),
            "w1t": w1t, "b1": b1c, "w2p": w2p, "b2t": b2t,
            "ft": ft, "ident": ident, "zeros": zeros, "rep": rep,
        })
    return in_maps


def kernel(x, w1, b1, w2, b2, base_filters):
    global _BUILT
    if _BUILT is None:
        _BUILT = _build()
    nc = _BUILT
    in_maps = _prep_inputs(np.asarray(x, dtype=np.float32),
                           np.asarray(w1, dtype=np.float32),
                           np.asarray(b1, dtype=np.float32),
                           np.asarray(w2, dtype=np.float32),
                           np.asarray(b2, dtype=np.float32),
                           np.asarray(base_filters, dtype=np.float32))
    res = run_bass_kernel_spmd(nc, in_maps, core_ids=list(range(N_CORES)))
    outs = []
    for core in range(N_CORES):
        o = np.asarray(res.results[core]["out"]).astype(np.float32)
        outs.append(o.reshape(BPC, CO, H, W))
    return np.concatenate(outs, axis=0).astype(np.float32)
